# revision 2
# baseline (speedup 1.0000x reference)
"""Trainium2 Bass kernel for the Alignment problem.

reference semantics (per batch):
    attn = (a @ b.T) * temperature                       # [LA, LB]
    mask = outer(mask_a, mask_b) > 0
    attn = where(mask, attn, -1e7)
    attn_a = softmax(attn, axis=0)   # over i (a-tokens)
    attn_b = softmax(attn, axis=1)   # over j (b-tokens)
    feature_b = attn_a.T @ a         # [LB, H]
    feature_a = attn_b @ b           # [LA, H]

Sharding: batch 16 -> 2 per core across 8 NeuronCores (pure data parallel).

Mask handling (exact-Z scheme, no mask matmuls on device):
The host pre-multiplies the masks into the operands: a~ = mask_a*a,
b~ = mask_b*b (bf16). Scores S~[i,j] = <a~_i, b~_j> are then exactly 0
whenever i or j is masked. The device computes ONE shared exp matrix
E = exp(temp*S~ + bias) with a constant bias (0 nominally); masked
entries become exp(bias) EXACTLY. Consequences:
  - feature matmuls use the pre-masked a~/b~ as rhs, so masked rows/cols
    contribute exactly 0 to the feature sums;
  - the softmax normalizers only need a scalar correction
    Z_valid = Z_accum - K*exp(bias), where K = #masked positions on the
    summed axis (a per-batch host constant, exact when bias = 0);
  - rows that are themselves masked come out wrong on device and are
    patched on the host (their reference value is the plain mean of the
    other operand's rows), as are any residual nonfinite rows (exact
    recompute safety net, off-nominal inputs only).

Because the shift is constant (not per-row), ONE matrix E serves BOTH
softmax directions; only the normalizers differ: Z_b[i] = row sums of E
(free ACT accumulator on the exp pass), Z_a[j] = row sums of E^T (free
ACT accumulator on the PSUM->SBUF copy after the PE transpose of E).

PE work per batch = 3 main matmuls (32768 cycles each) + E transpose
(8192) = 106496 cycles; both input transposes ride the DMA engines
(transposing descriptors), masks/temperature ride the exp activation.
"""

import sys

sys.path.insert(0, "/opt/trn_rl_repo")

import numpy as np

import concourse.bass as bass
import concourse.tile as tile
from concourse import mybir
from concourse.masks import make_identity
from concourse.bass_utils import run_bass_kernel_spmd

B, LA, LB, H = 16, 1024, 1024, 512
NCORES = 8
BPC = B // NCORES  # batches per core
P = 128

FP32 = mybir.dt.float32
BF16 = mybir.dt.bfloat16
AF = mybir.ActivationFunctionType

NI = LA // P  # 8 i row-blocks
NJ = LB // P  # 8 j row-blocks
KC = H // P  # 4 contraction chunks of the H axis
NJH = LB // 512  # j halves (psum-bank-sized)

# scalar-vector layout: [temp, bias, kb0, ka0, kb1, ka1]
NSC = 2 + 2 * BPC

POOL_SPECS = [
    ("nat", 2, None),
    ("tr", 2, None),
    ("esb", 2, None),
    ("etsb", 1, None),
    ("small", 1, None),
    ("stat", 2, None),
    ("outp", 1, None),
    ("ps_w", 2, "PSUM"),
    ("ps_score", 2, "PSUM"),
]


def emit_consts(nc, singles):
    identbf = singles.tile([P, P], BF16, tag="identbf", name="identbf")
    make_identity(nc, identbf[:])
    return dict(identbf=identbf)


def emit_loads(nc, pools, exts, bi):
    """Emit one batch's input DMAs, ordered along the startup critical
    path: transposed chunk pairs (aT, bT) first halves first, then second
    halves, then the natural-layout feature operands."""
    p_nat = pools["nat"]
    p_t = pools["tr"]
    a_ext, b_ext = exts["a"], exts["b"]

    aT = [p_t.tile([P, LA], BF16, tag=f"aT{c}", name=f"aT{c}") for c in range(KC)]
    bT = [p_t.tile([P, LB], BF16, tag=f"bT{c}", name=f"bT{c}") for c in range(KC)]
    # half-granularity transposing DMAs straight from DRAM: the first
    # score matmuls gate only on the h0 halves.
    for h in range(2):
        for c in range(KC):
            for t, ext in ((aT, a_ext), (bT, b_ext)):
                nc.sync.dma_start(
                    out=t[c][:, h * 512 : (h + 1) * 512],
                    in_=ext[bi, h * 512 : (h + 1) * 512, c * P : (c + 1) * P],
                    transpose=True,
                )

    An = [None] * NI
    Bn = [None] * NJ
    for names, ext, tag in ((An, a_ext, "An4_"), (Bn, b_ext, "Bn4_")):
        for rh in range(2):
            t = p_nat.tile([P, 4, H], BF16, tag=f"{tag}{rh}", name=f"{tag}{rh}")
            nc.sync.dma_start(
                out=t[:],
                in_=ext[bi, rh * 512 : (rh + 1) * 512, :].rearrange(
                    "(r p) d -> p r d", p=P
                ),
            )
            for q in range(4):
                names[rh * 4 + q] = t[:, q, :]
    return dict(aT=aT, bT=bT, An=An, Bn=Bn)


def emit_scores(nc, pools, loads, sc, bi):
    """Score matmuls + shared exp for one batch. Returns E tiles and the
    (uncorrected) row-sum accumulator zb_all [P, NI]."""
    p_e = pools["esb"]
    p_st = pools["stat"]
    p_ps_s = pools["ps_score"]
    aT, bT = loads["aT"], loads["bT"]

    E = []
    zb_all = p_st.tile([P, NI], FP32, tag=f"zb{bi}", name=f"zb{bi}")
    for ib in range(NI):
        s2 = p_ps_s.tile([P, LB], FP32, tag="score", name="score")
        for jh in range(NJH):
            seg = s2[:, jh * 512 : (jh + 1) * 512]
            for c in range(KC):
                nc.tensor.matmul(
                    seg,
                    lhsT=aT[c][:, ib * P : (ib + 1) * P],
                    rhs=bT[c][:, jh * 512 : (jh + 1) * 512],
                    start=(c == 0),
                    stop=(c == KC - 1),
                )
        e = p_e.tile([P, LB], BF16, tag=f"E{bi}_{ib}", name=f"E{ib}")
        nc.scalar.activation(
            out=e[:],
            in_=s2[:],
            func=AF.Exp,
            bias=sc[:, 1:2],
            scale=sc[:, 0:1],
            accum_out=zb_all[:, ib : ib + 1],
        )
        E.append(e)
    return E, zb_all


def emit_tail(nc, pools, consts, loads, E, zb_all, sc, exts, bi):
    """E transpose (+ Z_a accumulation), normalizer corrections, feature
    matmuls, normalize, and batched output DMAs for one batch."""
    p_et = pools["etsb"]
    p_st = pools["stat"]
    p_out = pools["outp"]
    p_ps_w = pools["ps_w"]
    identbf = consts["identbf"]
    An, Bn = loads["An"], loads["Bn"]
    oa_ext, ob_ext = exts["out_a"], exts["out_b"]

    # ---- E^T via PE transpose; Z_a free via ACT copy accumulators ----
    ET = []
    zap = p_st.tile([P, 2 * NJ], FP32, tag=f"zap{bi}", name=f"zap{bi}")
    for jb in range(NJ):
        et = p_et.tile([P, LA], BF16, tag=f"ET{jb}", name=f"ET{jb}")
        for hh in range(2):
            etp = p_ps_w.tile([P, 512], BF16, tag="etp", name="etp")
            for q in range(4):
                ib = hh * 4 + q
                nc.tensor.transpose(
                    etp[:, q * P : (q + 1) * P],
                    E[ib][:, jb * P : (jb + 1) * P],
                    identbf[:],
                )
            nc.scalar.activation(
                out=et[:, hh * 512 : (hh + 1) * 512],
                in_=etp[:],
                func=AF.Copy,
                bias=0.0,
                scale=1.0,
                accum_out=zap[:, 2 * jb + hh : 2 * jb + hh + 1],
            )
        ET.append(et)

    # ---- normalizer corrections: rz = 1/(Z - K*exp(bias)) ----
    kb, ka = sc[:, 2 + 2 * bi : 3 + 2 * bi], sc[:, 3 + 2 * bi : 4 + 2 * bi]
    zbc = p_st.tile([P, NI], FP32, tag=f"zbc{bi}", name=f"zbc{bi}")
    nc.vector.tensor_scalar_sub(zbc[:], zb_all[:], kb)
    rzb = p_st.tile([P, NI], FP32, tag=f"rzb{bi}", name=f"rzb{bi}")
    nc.vector.reciprocal(rzb[:], zbc[:])
    za = p_st.tile([P, NJ], FP32, tag=f"za{bi}", name=f"za{bi}")
    nc.vector.tensor_add(za[:], zap[:, 0 : 2 * NJ : 2], zap[:, 1 : 2 * NJ : 2])
    zac = p_st.tile([P, NJ], FP32, tag=f"zac{bi}", name=f"zac{bi}")
    nc.vector.tensor_scalar_sub(zac[:], za[:], ka)
    rza = p_st.tile([P, NJ], FP32, tag=f"rza{bi}", name=f"rza{bi}")
    nc.vector.reciprocal(rza[:], zac[:])

    # ---- feature matmuls + normalize + batched output DMAs ----
    def out_dma(ext, stage, rh):
        nc.sync.dma_start(
            out=ext[bi, rh * 512 : (rh + 1) * 512, :].rearrange(
                "(r p) d -> p r d", p=P
            ),
            in_=stage[:],
        )

    for jb in range(NJ):
        if jb % 4 == 0:
            ob4 = p_out.tile([P, 4, H], BF16, tag=f"ob4_{jb // 4}", name="ob4")
        f = p_ps_w.tile([P, H], FP32, tag="w512", name="w512")
        for ic in range(NI):
            nc.tensor.matmul(
                f[:],
                lhsT=E[ic][:, jb * P : (jb + 1) * P],
                rhs=An[ic],
                start=(ic == 0),
                stop=(ic == NI - 1),
            )
        nc.vector.tensor_scalar_mul(ob4[:, jb % 4, :], f[:], rza[:, jb : jb + 1])
        if jb % 4 == 3:
            out_dma(ob_ext, ob4, jb // 4)
    for ib in range(NI):
        if ib % 4 == 0:
            oa4 = p_out.tile([P, 4, H], BF16, tag=f"oa4_{ib // 4}", name="oa4")
        f = p_ps_w.tile([P, H], FP32, tag="w512", name="w512")
        for jc in range(NJ):
            nc.tensor.matmul(
                f[:],
                lhsT=ET[jc][:, ib * P : (ib + 1) * P],
                rhs=Bn[jc],
                start=(jc == 0),
                stop=(jc == NJ - 1),
            )
        nc.vector.tensor_scalar_mul(oa4[:, ib % 4, :], f[:], rzb[:, ib : ib + 1])
        if ib % 4 == 3:
            out_dma(oa_ext, oa4, ib // 4)


def emit_body(nc, pools, exts, consts):
    # broadcast scalar vector: [temp, bias, kb0, ka0, kb1, ka1]
    sc = pools["small"].tile([P, NSC], FP32, tag="sc", name="sc")
    nc.sync.dma_start(out=sc[:], in_=exts["scal"][:].to_broadcast([P, NSC]))

    loads = [emit_loads(nc, pools, exts, bi) for bi in range(BPC)]
    # PE order: b0 scores, b1 scores (hides b0's exp latency), b0 tail,
    # b1 tail (b1's exps finish during b0's feature matmuls).
    scored = [emit_scores(nc, pools, loads[bi], sc, bi) for bi in range(BPC)]
    for bi in range(BPC):
        E, zb_all = scored[bi]
        emit_tail(nc, pools, consts, loads[bi], E, zb_all, sc, exts, bi)


def declare_exts(nc):
    return dict(
        a=nc.declare_dram_parameter("a", [BPC, LA, H], BF16, isOutput=False),
        b=nc.declare_dram_parameter("b", [BPC, LB, H], BF16, isOutput=False),
        scal=nc.declare_dram_parameter("scal", [1, NSC], FP32, isOutput=False),
        out_a=nc.declare_dram_parameter("out_a", [BPC, LA, H], BF16, isOutput=True),
        out_b=nc.declare_dram_parameter("out_b", [BPC, LB, H], BF16, isOutput=True),
    )


def build_nc() -> bass.Bass:
    import contextlib

    nc = bass.Bass()
    exts = declare_exts(nc)
    with tile.TileContext(nc) as tc, contextlib.ExitStack() as ctx:
        singles = ctx.enter_context(tc.tile_pool(name="singles", bufs=1))
        pools = {
            name: ctx.enter_context(
                tc.tile_pool(name=name, bufs=bufs, space=space)
                if space
                else tc.tile_pool(name=name, bufs=bufs)
            )
            for name, bufs, space in POOL_SPECS
        }
        consts = emit_consts(nc, singles)
        emit_body(nc, pools, exts, consts)
    return nc


def legalize_waits(nc: bass.Bass, cap_default: int = 1, cap_evsem: int = 2):
    """Walrus in this toolchain accepts only one embedded sync-wait per TPB
    instruction. Hoist excess waits onto standalone InstEventSemaphore
    instructions (<=2 waits each) on the same engine, preceding the
    instruction, which preserves per-engine program-order semantics."""
    for f in nc.m.functions:
        for blk in f.blocks:
            new = []
            for inst in blk.instructions:
                si = inst.sync_info
                if (
                    si is not None
                    and si.on_wait
                    and not isinstance(inst, mybir.InstEventSemaphore)
                    and len(si.on_wait) > cap_default
                ):
                    waits = list(si.on_wait)
                    keep, extra = waits[:cap_default], waits[cap_default:]
                    while extra:
                        chunk, extra = extra[:cap_evsem], extra[cap_evsem:]
                        new.append(
                            mybir.InstEventSemaphore(
                                name=nc.get_next_instruction_name(),
                                engine=inst.engine,
                                ins=[],
                                outs=[],
                                sync_info=mybir.SyncInfo(on_wait=chunk, on_update=[]),
                            )
                        )
                    si.on_wait = keep
                new.append(inst)
            blk.instructions[:] = new


_NC = None
LAST = None  # BassKernelResults of the most recent run (for test harness)


def kernel(a, b, mask_a, mask_b, temperature):
    global _NC, LAST
    import ml_dtypes

    a = np.ascontiguousarray(np.asarray(a, dtype=np.float32))
    b = np.ascontiguousarray(np.asarray(b, dtype=np.float32))
    ma = np.asarray(mask_a).astype(np.float32).reshape(B, LA)
    mb = np.asarray(mask_b).astype(np.float32).reshape(B, LB)
    temp = float(np.asarray(temperature))

    # pre-masked operands: masked rows are exactly zero on device
    am = np.ascontiguousarray((a * ma[:, :, None]).astype(ml_dtypes.bfloat16))
    bm = np.ascontiguousarray((b * mb[:, :, None]).astype(ml_dtypes.bfloat16))

    if _NC is None:
        _NC = build_nc()
        legalize_waits(_NC)

    # Constant exp bias: 0 nominally (masked entries exp(0)=1 exactly, so
    # the Z corrections below are exact). For larger score scales, a
    # negative bias guards against fp32 exp overflow; the correction then
    # uses exp(bias), and any row whose Z underflows/overflows anyway is
    # exactly recomputed by the safety net below.
    sigma = temp * float(np.sqrt(H * max(a.var(), 1e-30) * max(b.var(), 1e-30)))
    bias_val = min(0.0, 80.0 - 6.5 * sigma)
    ecorr = float(np.exp(np.float64(bias_val)))

    in_maps = []
    for c in range(NCORES):
        sl = slice(c * BPC, (c + 1) * BPC)
        scal = np.zeros((1, NSC), np.float32)
        scal[0, 0] = temp
        scal[0, 1] = bias_val
        for bi in range(BPC):
            gb = c * BPC + bi
            scal[0, 2 + 2 * bi] = (LB - mb[gb].sum()) * ecorr  # K_b * e^bias
            scal[0, 3 + 2 * bi] = (LA - ma[gb].sum()) * ecorr  # K_a * e^bias
        in_maps.append({"a": am[sl], "b": bm[sl], "scal": scal})

    LAST = run_bass_kernel_spmd(_NC, in_maps, core_ids=list(range(NCORES)))
    feature_a = np.concatenate(
        [np.asarray(r["out_a"]).astype(np.float32) for r in LAST.results], axis=0
    )
    feature_b = np.concatenate(
        [np.asarray(r["out_b"]).astype(np.float32) for r in LAST.results], axis=0
    )

    # masked rows: reference softmaxes a constant row -> uniform -> plain
    # mean of the other operand's (raw) rows
    for bi in range(B):
        feature_a[bi, ma[bi] == 0.0, :] = b[bi].mean(axis=0)
        feature_b[bi, mb[bi] == 0.0, :] = a[bi].mean(axis=0)

    # safety net: exactly recompute any residual nonfinite rows (e.g. Z
    # underflow under off-nominal score scales). Nominal inputs never
    # trigger this; the check itself is a cheap scan.
    def _fix_rows(feat, this, other, row_mask, col_mask):
        bad_b, bad_r = np.nonzero(~np.isfinite(feat).all(axis=2))
        for bi, r in zip(bad_b, bad_r):
            srow = (other[bi] @ this[bi, r]) * temp  # scores vs. all others
            srow = np.where(
                (row_mask[bi, r] * col_mask[bi]) > 0, srow, -1e7
            ).astype(np.float64)
            srow -= srow.max()
            w = np.exp(srow)
            w /= w.sum()
            feat[bi, r, :] = (w @ other[bi]).astype(np.float32)

    if not np.isfinite(feature_a).all() or not np.isfinite(feature_b).all():
        _fix_rows(feature_a, a, b, ma, mb)
        _fix_rows(feature_b, b, a, mb, ma)
    return feature_a, feature_b


# revision 14
# speedup vs baseline: 1.2713x; 1.2713x over previous
"""Trainium2 Bass kernel for the Alignment problem.

reference semantics (per batch):
    attn = (a @ b.T) * temperature                       # [LA, LB]
    mask = outer(mask_a, mask_b) > 0
    attn = where(mask, attn, -1e7)
    attn_a = softmax(attn, axis=0)   # over i (a-tokens)
    attn_b = softmax(attn, axis=1)   # over j (b-tokens)
    feature_b = attn_a.T @ a         # [LB, H]
    feature_a = attn_b @ b           # [LA, H]

Sharding: batch 16 -> 2 per core across 8 NeuronCores (pure data parallel).

Mask handling (exact-Z scheme, no mask matmuls on device):
The host pre-multiplies the masks into the operands: a~ = mask_a*a,
b~ = mask_b*b (bf16). Scores S~[i,j] = <a~_i, b~_j> are then exactly 0
whenever i or j is masked. The device computes ONE shared exp matrix
E = exp(temp*S~ + bias) with a constant bias (0 nominally); masked
entries become exp(bias) EXACTLY. Consequences:
  - feature matmuls use the pre-masked a~/b~ as rhs, so masked rows/cols
    contribute exactly 0 to the feature sums;
  - the softmax normalizers only need a scalar correction
    Z_valid = Z_accum - K*exp(bias), where K = #masked positions on the
    summed axis (a per-batch host constant, exact when bias = 0);
  - rows that are themselves masked come out wrong on device and are
    patched on the host (their reference value is the plain mean of the
    other operand's rows), as are any residual nonfinite rows (exact
    recompute safety net, off-nominal inputs only).

Because the shift is constant (not per-row), ONE matrix E serves BOTH
softmax directions; only the normalizers differ: Z_b[i] = row sums of E
(free ACT accumulator on the exp pass), Z_a[j] = row sums of E^T (free
ACT accumulator on the PSUM->SBUF copy after the PE transpose of E).

PE work per batch = 3 main matmuls (32768 cycles each) + E transpose
(8192) = 106496 cycles; both input transposes ride the DMA engines
(transposing descriptors), masks/temperature ride the exp activation.
"""

import sys

sys.path.insert(0, "/opt/trn_rl_repo")

import numpy as np

import concourse.bass as bass
import concourse.tile as tile
from concourse import mybir
from concourse.masks import make_identity
from concourse.bass_utils import run_bass_kernel_spmd

B, LA, LB, H = 16, 1024, 1024, 512
NCORES = 8
BPC = B // NCORES  # batches per core
P = 128

FP32 = mybir.dt.float32
BF16 = mybir.dt.bfloat16
AF = mybir.ActivationFunctionType

NI = LA // P  # 8 i row-blocks
NJ = LB // P  # 8 j row-blocks
KC = H // P  # 4 contraction chunks of the H axis
NJH = LB // 512  # j halves (psum-bank-sized)

# scalar-vector layout: [kb0, ka0, kb1, ka1] (Z corrections; per-core
# runtime values -- temp/bias are baked as immediates, see build_nc)
NSC = 2 * BPC

POOL_SPECS = [
    ("nat", 2, None),
    ("tr", 2, None),
    ("esb", 2, None),
    ("etsb", 1, None),
    ("small", 1, None),
    ("stat", 2, None),
    ("outp", 1, None),
    ("ps_w", 2, "PSUM"),
    ("ps_score", 2, "PSUM"),
]


def emit_consts(nc, singles):
    identbf = singles.tile([P, P], BF16, tag="identbf", name="identbf")
    make_identity(nc, identbf[:])
    return dict(identbf=identbf)


def emit_transposes(nc, pools, exts, bi):
    """Transposing DMAs (HWDGE) for the score-matmul operands, full
    height, chunk-interleaved so the first score matmuls gate on the
    first chunk pair only."""
    p_t = pools["tr"]
    a_ext, b_ext = exts["a"], exts["b"]
    aT = [p_t.tile([P, LA], BF16, tag=f"aT{c}", name=f"aT{c}") for c in range(KC)]
    bT = [p_t.tile([P, LB], BF16, tag=f"bT{c}", name=f"bT{c}") for c in range(KC)]

    def tr(t, ext, c, h):
        nc.sync.dma_start(
            out=t[c][:, h * 512 : (h + 1) * 512],
            in_=ext[bi, h * 512 : (h + 1) * 512, c * P : (c + 1) * P],
            transpose=True,
        )

    # arrival order mirrors first-use order: (aT,bT) h0 chunk pairs gate
    # the ib=0/jh=0 segment, bT h1 gates jh=1, aT h1 gates ib>=4
    for c in range(KC):
        tr(aT, a_ext, c, 0)
        tr(bT, b_ext, c, 0)
    for c in range(KC):
        tr(bT, b_ext, c, 1)
    for c in range(KC):
        tr(aT, a_ext, c, 1)
    return dict(aT=aT, bT=bT)


def emit_nat_loads(nc, pools, exts, bi, loads):
    """Natural-layout feature-matmul operands. Issued on SP AFTER all
    transposes so program order keeps the (serialized) DMA engines free
    for the score-critical transposed loads first."""
    p_nat = pools["nat"]
    An = [None] * NI
    Bn = [None] * NJ
    for names, ext, tag in ((An, exts["a"], "An4_"), (Bn, exts["b"], "Bn4_")):
        for rh in range(2):
            t = p_nat.tile([P, 4, H], BF16, tag=f"{tag}{rh}", name=f"{tag}{rh}")
            nc.sync.dma_start(
                out=t[:],
                in_=ext[bi, rh * 512 : (rh + 1) * 512, :].rearrange(
                    "(r p) d -> p r d", p=P
                ),
            )
            for q in range(4):
                names[rh * 4 + q] = t[:, q, :]
    loads["An"] = An
    loads["Bn"] = Bn


def emit_scores(nc, pools, loads, scale_bias, bi):
    """Score matmuls + shared exp for one batch. Returns E tiles and the
    (uncorrected) row-sum accumulator zb_all [P, NI]."""
    temp_imm, bias_imm = scale_bias
    p_e = pools["esb"]
    p_st = pools["stat"]
    p_ps_s = pools["ps_score"]
    aT, bT = loads["aT"], loads["bT"]

    E = []
    zb_all = p_st.tile([P, NI], FP32, tag=f"zb{bi}", name=f"zb{bi}")
    for ib in range(NI):
        s2 = p_ps_s.tile([P, LB], FP32, tag="score", name="score")
        for jh in range(NJH):
            seg = s2[:, jh * 512 : (jh + 1) * 512]
            for c in range(KC):
                nc.tensor.matmul(
                    seg,
                    lhsT=aT[c][:, ib * P : (ib + 1) * P],
                    rhs=bT[c][:, jh * 512 : (jh + 1) * 512],
                    start=(c == 0),
                    stop=(c == KC - 1),
                )
        e = p_e.tile([P, LB], BF16, tag=f"E{bi}_{ib}", name=f"E{ib}")
        nc.scalar.activation(
            out=e[:],
            in_=s2[:],
            func=AF.Exp,
            bias=bias_imm,
            scale=temp_imm,
            accum_out=zb_all[:, ib : ib + 1],
        )
        E.append(e)
    return E, zb_all


def emit_tail(nc, pools, consts, loads, E, zb_all, sc, exts, bi):
    """E transpose (+ Z_a accumulation), normalizer corrections, feature
    matmuls, normalize, and batched output DMAs for one batch."""
    p_et = pools["etsb"]
    p_st = pools["stat"]
    p_out = pools["outp"]
    p_ps_w = pools["ps_w"]
    identbf = consts["identbf"]
    An, Bn = loads["An"], loads["Bn"]
    oa_ext, ob_ext = exts["out_a"], exts["out_b"]

    # ---- normalizer correction (b-direction): rz = 1/(Z - K*e^bias) ----
    kb, ka = sc[:, 2 * bi : 2 * bi + 1], sc[:, 2 * bi + 1 : 2 * bi + 2]
    zbc = p_st.tile([P, NI], FP32, tag=f"zbc{bi}", name=f"zbc{bi}")
    nc.vector.tensor_scalar_sub(zbc[:], zb_all[:], kb)
    rzb = p_st.tile([P, NI], FP32, tag=f"rzb{bi}", name=f"rzb{bi}")
    nc.vector.reciprocal(rzb[:], zbc[:])

    def out_dma(ext, stage, g):
        nc.sync.dma_start(
            out=ext[bi, g * 256 : (g + 1) * 256, :].rearrange(
                "(r p) d -> p r d", p=P
            ),
            in_=stage[:],
        )

    # ---- E^T (PE transpose; Z_a free via ACT copy accumulators)
    # interleaved with the f_b feature matmuls, so the ACT copy latency
    # hides under PE's feature work and PE never waits on the etp pool.
    # Z_a for block jb needs only this block's two accumulators, so its
    # correction + reciprocal run per-block on the (idle) DVE.
    ET = []
    zap = p_st.tile([P, 2 * NJ], FP32, tag=f"zap{bi}", name=f"zap{bi}")
    za = p_st.tile([P, NJ], FP32, tag=f"za{bi}", name=f"za{bi}")
    zac = p_st.tile([P, NJ], FP32, tag=f"zac{bi}", name=f"zac{bi}")
    rza = p_st.tile([P, NJ], FP32, tag=f"rza{bi}", name=f"rza{bi}")
    for jb in range(NJ):
        et = p_et.tile([P, LA], BF16, tag=f"ET{jb}", name=f"ET{jb}")
        for hh in range(2):
            etp = p_ps_w.tile([P, 512], BF16, tag="etp", name="etp")
            for q in range(4):
                ib = hh * 4 + q
                nc.tensor.transpose(
                    etp[:, q * P : (q + 1) * P],
                    E[ib][:, jb * P : (jb + 1) * P],
                    identbf[:],
                )
            nc.scalar.activation(
                out=et[:, hh * 512 : (hh + 1) * 512],
                in_=etp[:],
                func=AF.Copy,
                bias=0.0,
                scale=1.0,
                accum_out=zap[:, 2 * jb + hh : 2 * jb + hh + 1],
            )
        ET.append(et)
        nc.vector.tensor_add(
            za[:, jb : jb + 1],
            zap[:, 2 * jb : 2 * jb + 1],
            zap[:, 2 * jb + 1 : 2 * jb + 2],
        )
        nc.vector.tensor_scalar_sub(zac[:, jb : jb + 1], za[:, jb : jb + 1], ka)
        nc.vector.reciprocal(rza[:, jb : jb + 1], zac[:, jb : jb + 1])

        if jb % 2 == 0:
            ob2 = p_out.tile([P, 2, H], BF16, tag=f"ob2_{jb // 2}", name="ob2")
        f = p_ps_w.tile([P, H], FP32, tag="w512", name="w512")
        for ic in range(NI):
            nc.tensor.matmul(
                f[:],
                lhsT=E[ic][:, jb * P : (jb + 1) * P],
                rhs=An[ic],
                start=(ic == 0),
                stop=(ic == NI - 1),
            )
        nc.vector.tensor_scalar_mul(ob2[:, jb % 2, :], f[:], rza[:, jb : jb + 1])
        if jb % 2 == 1:
            out_dma(ob_ext, ob2, jb // 2)

    for ib in range(NI):
        if ib % 2 == 0:
            oa2 = p_out.tile([P, 2, H], BF16, tag=f"oa2_{ib // 2}", name="oa2")
        f = p_ps_w.tile([P, H], FP32, tag="w512", name="w512")
        for jc in range(NJ):
            nc.tensor.matmul(
                f[:],
                lhsT=ET[jc][:, ib * P : (ib + 1) * P],
                rhs=Bn[jc],
                start=(jc == 0),
                stop=(jc == NJ - 1),
            )
        nc.vector.tensor_scalar_mul(oa2[:, ib % 2, :], f[:], rzb[:, ib : ib + 1])
        if ib % 2 == 1:
            out_dma(oa_ext, oa2, ib // 2)


def emit_body(nc, pools, exts, consts, scale_bias):
    # DMA order = need order: b0 transposes, b1 transposes, natural-layout
    # operands, then the (late-needed) Z-correction scalars.
    loads = [emit_transposes(nc, pools, exts, bi) for bi in range(BPC)]
    for bi in range(BPC):
        emit_nat_loads(nc, pools, exts, bi, loads[bi])
    sc = pools["small"].tile([P, NSC], FP32, tag="sc", name="sc")
    nc.sync.dma_start(out=sc[:], in_=exts["scal"][:].to_broadcast([P, NSC]))
    # PE order: b0 scores, b1 scores (hides b0's exp latency), b0 tail,
    # b1 tail (b1's exps finish during b0's feature matmuls).
    scored = [emit_scores(nc, pools, loads[bi], scale_bias, bi) for bi in range(BPC)]
    for bi in range(BPC):
        E, zb_all = scored[bi]
        emit_tail(nc, pools, consts, loads[bi], E, zb_all, sc, exts, bi)


def declare_exts(nc):
    return dict(
        a=nc.declare_dram_parameter("a", [BPC, LA, H], BF16, isOutput=False),
        b=nc.declare_dram_parameter("b", [BPC, LB, H], BF16, isOutput=False),
        scal=nc.declare_dram_parameter("scal", [1, NSC], FP32, isOutput=False),
        out_a=nc.declare_dram_parameter("out_a", [BPC, LA, H], BF16, isOutput=True),
        out_b=nc.declare_dram_parameter("out_b", [BPC, LB, H], BF16, isOutput=True),
    )


def build_nc(scale_bias=(1.0, 0.0)) -> bass.Bass:
    import contextlib

    nc = bass.Bass()
    exts = declare_exts(nc)
    with tile.TileContext(nc) as tc, contextlib.ExitStack() as ctx:
        singles = ctx.enter_context(tc.tile_pool(name="singles", bufs=1))
        pools = {
            name: ctx.enter_context(
                tc.tile_pool(name=name, bufs=bufs, space=space)
                if space
                else tc.tile_pool(name=name, bufs=bufs)
            )
            for name, bufs, space in POOL_SPECS
        }
        consts = emit_consts(nc, singles)
        emit_body(nc, pools, exts, consts, scale_bias)
    return nc


def legalize_waits(nc: bass.Bass, cap_default: int = 1, cap_evsem: int = 2):
    """Walrus in this toolchain accepts only one embedded sync-wait per TPB
    instruction. Hoist excess waits onto standalone InstEventSemaphore
    instructions (<=2 waits each) on the same engine, preceding the
    instruction, which preserves per-engine program-order semantics."""
    for f in nc.m.functions:
        for blk in f.blocks:
            new = []
            for inst in blk.instructions:
                si = inst.sync_info
                if (
                    si is not None
                    and si.on_wait
                    and not isinstance(inst, mybir.InstEventSemaphore)
                    and len(si.on_wait) > cap_default
                ):
                    waits = list(si.on_wait)
                    keep, extra = waits[:cap_default], waits[cap_default:]
                    while extra:
                        chunk, extra = extra[:cap_evsem], extra[cap_evsem:]
                        new.append(
                            mybir.InstEventSemaphore(
                                name=nc.get_next_instruction_name(),
                                engine=inst.engine,
                                ins=[],
                                outs=[],
                                sync_info=mybir.SyncInfo(on_wait=chunk, on_update=[]),
                            )
                        )
                    si.on_wait = keep
                new.append(inst)
            blk.instructions[:] = new


_NC = None
_NC_KEY = None
LAST = None  # BassKernelResults of the most recent run (for test harness)


def kernel(a, b, mask_a, mask_b, temperature):
    global _NC, LAST
    import ml_dtypes

    a = np.ascontiguousarray(np.asarray(a, dtype=np.float32))
    b = np.ascontiguousarray(np.asarray(b, dtype=np.float32))
    ma = np.asarray(mask_a).astype(np.float32).reshape(B, LA)
    mb = np.asarray(mask_b).astype(np.float32).reshape(B, LB)
    temp = float(np.asarray(temperature))

    # pre-masked operands: masked rows are exactly zero on device
    am = np.ascontiguousarray((a * ma[:, :, None]).astype(ml_dtypes.bfloat16))
    bm = np.ascontiguousarray((b * mb[:, :, None]).astype(ml_dtypes.bfloat16))

    # Constant exp bias: 0 nominally (masked entries exp(0)=1 exactly, so
    # the Z corrections below are exact). For larger score scales, a
    # negative bias guards against fp32 exp overflow; the correction then
    # uses exp(bias), and any row whose Z underflows/overflows anyway is
    # exactly recomputed by the safety net below.
    sigma = temp * float(np.sqrt(H * max(a.var(), 1e-30) * max(b.var(), 1e-30)))
    bias_val = min(0.0, 80.0 - 6.5 * sigma)
    ecorr = float(np.exp(np.float64(bias_val)))

    global _NC_KEY
    if _NC is None or _NC_KEY != (temp, bias_val):
        _NC = build_nc((temp, bias_val))
        legalize_waits(_NC)
        _NC_KEY = (temp, bias_val)

    in_maps = []
    for c in range(NCORES):
        sl = slice(c * BPC, (c + 1) * BPC)
        scal = np.zeros((1, NSC), np.float32)
        for bi in range(BPC):
            gb = c * BPC + bi
            scal[0, 2 * bi] = (LB - mb[gb].sum()) * ecorr  # K_b * e^bias
            scal[0, 2 * bi + 1] = (LA - ma[gb].sum()) * ecorr  # K_a * e^bias
        in_maps.append({"a": am[sl], "b": bm[sl], "scal": scal})

    LAST = run_bass_kernel_spmd(_NC, in_maps, core_ids=list(range(NCORES)))
    feature_a = np.concatenate(
        [np.asarray(r["out_a"]).astype(np.float32) for r in LAST.results], axis=0
    )
    feature_b = np.concatenate(
        [np.asarray(r["out_b"]).astype(np.float32) for r in LAST.results], axis=0
    )

    # masked rows: reference softmaxes a constant row -> uniform -> plain
    # mean of the other operand's (raw) rows
    for bi in range(B):
        feature_a[bi, ma[bi] == 0.0, :] = b[bi].mean(axis=0)
        feature_b[bi, mb[bi] == 0.0, :] = a[bi].mean(axis=0)

    # safety net: exactly recompute any residual nonfinite rows (e.g. Z
    # underflow under off-nominal score scales). Nominal inputs never
    # trigger this; the check itself is a cheap scan.
    def _fix_rows(feat, this, other, row_mask, col_mask):
        bad_b, bad_r = np.nonzero(~np.isfinite(feat).all(axis=2))
        for bi, r in zip(bad_b, bad_r):
            srow = (other[bi] @ this[bi, r]) * temp  # scores vs. all others
            srow = np.where(
                (row_mask[bi, r] * col_mask[bi]) > 0, srow, -1e7
            ).astype(np.float64)
            srow -= srow.max()
            w = np.exp(srow)
            w /= w.sum()
            feat[bi, r, :] = (w @ other[bi]).astype(np.float32)

    if not np.isfinite(feature_a).all() or not np.isfinite(feature_b).all():
        _fix_rows(feature_a, a, b, ma, mb)
        _fix_rows(feature_b, b, a, mb, ma)
    return feature_a, feature_b
